# revision 9
# baseline (speedup 1.0000x reference)
"""Trainium2 Bass kernel for nn_Conv2d_45810121179422.

Conv2d: x(32,128,56,56) f32, weight(256,128,3,3), bias(256), stride 1, pad 1
-> out(32,256,56,56) f32.

Strategy: data-parallel over batch across 8 NeuronCores (4 images/core).
Per core, an implicit-GEMM conv: input channels (128) live on the SBUF
partition dim, the 3x3 conv becomes 9 accumulating matmuls into PSUM with
spatially shifted views of a zero-padded input, weights are the stationary
operand (one [128ic, 128oc] slab per (kh, kw, oc-half)). Bias is fused into
the PSUM->SBUF drain on the Scalar engine.

Matmuls run in fp32r (TF32-like 1s+8e+11m, full PE rate); operands are
pre-rounded to 11 mantissa bits on the host so hardware truncation is exact.
Measured ~1.3e-4 max rel err vs the fp32 reference.

DMA layout: x loads stream on the Sync HWDGE queue (img0 split into two row
bands so compute starts early), weights/bias + all output stores use the
Scalar HWDGE queue — loads and stores never queue behind each other.
"""

import numpy as np

import concourse.bass as bass
import concourse.tile as tile
from concourse import bacc, mybir
from concourse.bass_utils import run_bass_kernel_spmd

# Problem constants (hardcoded per harness contract)
N, IN_C, H, W = 32, 128, 56, 56
OUT_C, K, PAD = 256, 3, 1
N_CORES = 8
IMGS = N // N_CORES          # 4 images per core
HP, WP = H + 2 * PAD, W + 2 * PAD  # 58, 58 padded
ROWS_PER_TILE = 8            # output rows per matmul group (free dim 8*56=448)
N_CHUNKS = H // ROWS_PER_TILE  # 7
FREE = ROWS_PER_TILE * W     # 448
HALVES = OUT_C // 128        # 2
HW_ = H * W                  # 3136
X_SPLIT = 30                 # padded-row split for the img0 load (covers chunks 0-2)

MM_MODE = "f32r"


def build_nc(mm_mode: str = MM_MODE):
    f32 = mybir.dt.float32
    mm_dt = {"f32r": mybir.dt.float32r, "f32": f32}[mm_mode]

    nc = bacc.Bacc("TRN2", target_bir_lowering=False, debug=False)

    xp = nc.dram_tensor("xp", [IN_C, IMGS, HP, WP], mm_dt, kind="ExternalInput").ap()
    wt = nc.dram_tensor(
        "wt", [IN_C, HALVES, K * K, 128], mm_dt, kind="ExternalInput"
    ).ap()
    bs = nc.dram_tensor("bs", [128, HALVES], f32, kind="ExternalInput").ap()
    out = nc.dram_tensor(
        "out", [HALVES, 128, IMGS, HW_], f32, kind="ExternalOutput"
    ).ap()

    with tile.TileContext(nc) as tc:
        with (
            tc.tile_pool(name="consts", bufs=1) as consts,
            tc.tile_pool(name="psum", bufs=8, space="PSUM") as psum,
            tc.tile_pool(name="outp", bufs=6) as outp,
        ):
            x_sb = consts.tile([IN_C, IMGS, HP, WP], mm_dt)
            w_sb = consts.tile([IN_C, HALVES, K * K, 128], mm_dt)
            b_sb = consts.tile([128, HALVES], f32)

            # All loads on the Sync HWDGE queue, critical-path first: the
            # queues share the 16 SDMA engines (aggregate-BW-bound), so
            # priority ordering on one queue beats splitting across two.
            # First compute group needs w_h0 + x img0 rows 0..17 only.
            nc.scalar.dma_start(out=b_sb[:], in_=bs)
            # First compute group needs only w[half0, kh=0] + x img0 rows 0..9.
            nc.sync.dma_start(out=w_sb[:, 0, :K], in_=wt[:, 0, :K])
            nc.sync.dma_start(out=x_sb[:, 0, :10], in_=xp[:, 0, :10])
            nc.sync.dma_start(out=w_sb[:, 0, K:], in_=wt[:, 0, K:])
            nc.sync.dma_start(out=x_sb[:, 0, 10:18], in_=xp[:, 0, 10:18])
            nc.sync.dma_start(out=w_sb[:, 1], in_=wt[:, 1])
            nc.sync.dma_start(out=x_sb[:, 0, 18:42], in_=xp[:, 0, 18:42])
            nc.sync.dma_start(out=x_sb[:, 0, 42:], in_=xp[:, 0, 42:])
            for img in range(1, IMGS):
                nc.sync.dma_start(out=x_sb[:, img], in_=xp[:, img])

            for img in range(IMGS):
                for half in range(HALVES):
                    for chunk in range(N_CHUNKS):
                        r0 = chunk * ROWS_PER_TILE
                        ps = psum.tile([128, FREE], f32)
                        i = 0
                        for kh in range(K):
                            for kw in range(K):
                                rhs = x_sb[
                                    :, img, r0 + kh : r0 + kh + ROWS_PER_TILE,
                                    kw : kw + W,
                                ]
                                lhsT = w_sb[:, half, kh * K + kw, :]
                                nc.tensor.matmul(
                                    ps[:],
                                    lhsT,
                                    rhs,
                                    start=(i == 0),
                                    stop=(i == K * K - 1),
                                )
                                i += 1
                        o_sb = outp.tile([128, FREE], f32)
                        nc.scalar.activation(
                            out=o_sb[:],
                            in_=ps[:],
                            func=mybir.ActivationFunctionType.Identity,
                            bias=b_sb[:, half : half + 1],
                            scale=1.0,
                        )
                        # Alternate store queues; late stores prefer the
                        # sync queue (idle once loads finish) so the tail
                        # store isn't queued behind earlier ones.
                        st_eng = nc.sync if (chunk % 2 == 1 or img >= 2) else nc.scalar
                        st_eng.dma_start(
                            out=out[half, :, img, r0 * W : (r0 + ROWS_PER_TILE) * W],
                            in_=o_sb[:],
                        )

    nc.compile()
    return nc


def round_fp32r(a: np.ndarray) -> np.ndarray:
    """Round fp32 to the PE's fp32r format (11 mantissa bits), RNE."""
    bits = np.ascontiguousarray(a, dtype=np.float32).view(np.uint32)
    lsb = (bits >> 12) & 1
    rounded = (bits + 0x7FF + lsb) & 0xFFFFF000
    return rounded.view(np.float32)


def shard_inputs(x: np.ndarray, weight: np.ndarray, bias: np.ndarray):
    """Host-side: pad + layout-transform into per-core in_maps."""
    x = np.ascontiguousarray(x, dtype=np.float32)
    weight = np.asarray(weight, dtype=np.float32)
    if MM_MODE == "f32r":
        x = round_fp32r(x)
        weight = round_fp32r(weight)
    # [core, C, img, HP, WP] zero-padded
    xp = np.zeros((N_CORES, IN_C, IMGS, HP, WP), dtype=np.float32)
    xt = x.reshape(N_CORES, IMGS, IN_C, H, W).transpose(0, 2, 1, 3, 4)
    xp[:, :, :, PAD : PAD + H, PAD : PAD + W] = xt
    # weight (OUT_C, IN_C, K, K) -> [IN_C, HALVES, K*K, 128]
    wt = np.ascontiguousarray(
        weight.transpose(1, 2, 3, 0)           # [IN_C, K, K, OUT_C]
        .reshape(IN_C, K * K, HALVES, 128)
        .transpose(0, 2, 1, 3)                 # [IN_C, HALVES, K*K, 128]
    )
    # bias (256,) -> [128, 2] with bs[p, half] = bias[half*128 + p]
    bs = np.ascontiguousarray(
        np.asarray(bias, dtype=np.float32).reshape(HALVES, 128).T
    )
    return [
        {"xp": np.ascontiguousarray(xp[c]), "wt": wt, "bs": bs}
        for c in range(N_CORES)
    ]


def unshard_output(results):
    """[core][out: (2,128,4,3136)] -> (32,256,56,56)."""
    o = np.stack([r["out"] for r in results])  # [8, 2, 128, 4, 3136]
    return np.ascontiguousarray(
        o.transpose(0, 3, 1, 2, 4).reshape(N, OUT_C, H, W)
    )


def kernel(x: np.ndarray, weight: np.ndarray, bias: np.ndarray) -> np.ndarray:
    nc = build_nc()
    in_maps = shard_inputs(x, weight, bias)
    res = run_bass_kernel_spmd(nc, in_maps, core_ids=list(range(N_CORES)))
    return unshard_output(res.results)


# revision 12
# speedup vs baseline: 1.0936x; 1.0936x over previous
"""Trainium2 Bass kernel for nn_Conv2d_45810121179422.

Conv2d: x(32,128,56,56) f32, weight(256,128,3,3), bias(256), stride 1, pad 1
-> out(32,256,56,56) f32.

Strategy: data-parallel over batch across 8 NeuronCores (4 images/core).
Per core, an implicit-GEMM conv: input channels (128) live on the SBUF
partition dim, the 3x3 conv becomes 9 accumulating matmuls into PSUM with
spatially shifted views of a zero-padded input, weights are the stationary
operand (one [128ic, 128oc] slab per (kh, kw, oc-half)). Bias is fused into
the PSUM->SBUF drain on the Scalar engine.

Matmuls run in fp32r (TF32-like 1s+8e+11m, full PE rate); operands are
pre-rounded to 11 mantissa bits on the host so hardware truncation is exact.
Measured ~1.3e-4 max rel err vs the fp32 reference.

DMA layout: x loads stream on the Sync HWDGE queue (img0 split into two row
bands so compute starts early), weights/bias + all output stores use the
Scalar HWDGE queue — loads and stores never queue behind each other.
"""

import numpy as np

import concourse.bass as bass
import concourse.tile as tile
from concourse import bacc, mybir
from concourse.bass_utils import run_bass_kernel_spmd

# Problem constants (hardcoded per harness contract)
N, IN_C, H, W = 32, 128, 56, 56
OUT_C, K, PAD = 256, 3, 1
N_CORES = 8
IMGS = N // N_CORES          # 4 images per core
HP, WP = H + 2 * PAD, W + 2 * PAD  # 58, 58 padded
ROWS_PER_TILE = 8            # output rows per matmul group (free dim 8*56=448)
N_CHUNKS = H // ROWS_PER_TILE  # 7
FREE = ROWS_PER_TILE * W     # 448
HALVES = OUT_C // 128        # 2
HW_ = H * W                  # 3136
X_SPLIT = 30                 # padded-row split for the img0 load (covers chunks 0-2)

import os

MM_MODE = os.environ.get("CONV_MM_MODE", "f32r")


def build_nc(mm_mode: str | None = None):
    mm_mode = mm_mode or MM_MODE
    f32 = mybir.dt.float32
    mm_dt = {
        "f32r": mybir.dt.float32r,
        "f32": f32,
        "bf16": mybir.dt.bfloat16,
    }[mm_mode]

    nc = bacc.Bacc("TRN2", target_bir_lowering=False, debug=False)

    xp = nc.dram_tensor("xp", [IN_C, IMGS, HP, WP], mm_dt, kind="ExternalInput").ap()
    wt = nc.dram_tensor(
        "wt", [IN_C, HALVES, K * K, 128], mm_dt, kind="ExternalInput"
    ).ap()
    bs = nc.dram_tensor("bs", [128, HALVES], f32, kind="ExternalInput").ap()
    out = nc.dram_tensor(
        "out", [HALVES, 128, IMGS, HW_], f32, kind="ExternalOutput"
    ).ap()

    with tile.TileContext(nc) as tc:
        with (
            tc.tile_pool(name="consts", bufs=1) as consts,
            tc.tile_pool(name="psum", bufs=8, space="PSUM") as psum,
            tc.tile_pool(name="outp", bufs=6) as outp,
        ):
            x_sb = consts.tile([IN_C, IMGS, HP, WP], mm_dt)
            w_sb = consts.tile([IN_C, HALVES, K * K, 128], mm_dt)
            b_sb = consts.tile([128, HALVES], f32)

            # All loads on the Sync HWDGE queue, critical-path first: the
            # queues share the 16 SDMA engines (aggregate-BW-bound), so
            # priority ordering on one queue beats splitting across two.
            # First compute group needs w_h0 + x img0 rows 0..17 only.
            nc.scalar.dma_start(out=b_sb[:], in_=bs)
            # First compute group needs only w[half0, kh=0] + x img0 rows 0..9.
            nc.sync.dma_start(out=w_sb[:, 0, :K], in_=wt[:, 0, :K])
            nc.sync.dma_start(out=x_sb[:, 0, :10], in_=xp[:, 0, :10])
            nc.sync.dma_start(out=w_sb[:, 0, K:], in_=wt[:, 0, K:])
            nc.sync.dma_start(out=x_sb[:, 0, 10:18], in_=xp[:, 0, 10:18])
            nc.sync.dma_start(out=w_sb[:, 1], in_=wt[:, 1])
            nc.sync.dma_start(out=x_sb[:, 0, 18:42], in_=xp[:, 0, 18:42])
            nc.sync.dma_start(out=x_sb[:, 0, 42:], in_=xp[:, 0, 42:])
            for img in range(1, IMGS):
                nc.sync.dma_start(out=x_sb[:, img], in_=xp[:, img])

            for img in range(IMGS):
                for half in range(HALVES):
                    for chunk in range(N_CHUNKS):
                        r0 = chunk * ROWS_PER_TILE
                        ps = psum.tile([128, FREE], f32)
                        i = 0
                        for kh in range(K):
                            for kw in range(K):
                                rhs = x_sb[
                                    :, img, r0 + kh : r0 + kh + ROWS_PER_TILE,
                                    kw : kw + W,
                                ]
                                lhsT = w_sb[:, half, kh * K + kw, :]
                                nc.tensor.matmul(
                                    ps[:],
                                    lhsT,
                                    rhs,
                                    start=(i == 0),
                                    stop=(i == K * K - 1),
                                )
                                i += 1
                        o_sb = outp.tile([128, FREE], f32)
                        nc.scalar.activation(
                            out=o_sb[:],
                            in_=ps[:],
                            func=mybir.ActivationFunctionType.Identity,
                            bias=b_sb[:, half : half + 1],
                            scale=1.0,
                        )
                        # Alternate store queues; late stores prefer the
                        # sync queue (idle once loads finish) so the tail
                        # store isn't queued behind earlier ones.
                        st_eng = nc.sync if (chunk % 2 == 1 or img >= 2) else nc.scalar
                        st_eng.dma_start(
                            out=out[half, :, img, r0 * W : (r0 + ROWS_PER_TILE) * W],
                            in_=o_sb[:],
                        )

    nc.compile()
    return nc


def round_fp32r(a: np.ndarray) -> np.ndarray:
    """Round fp32 to the PE's fp32r format (11 mantissa bits), RNE."""
    bits = np.ascontiguousarray(a, dtype=np.float32).view(np.uint32)
    lsb = (bits >> 12) & 1
    rounded = (bits + 0x7FF + lsb) & 0xFFFFF000
    return rounded.view(np.float32)


def shard_inputs(x: np.ndarray, weight: np.ndarray, bias: np.ndarray):
    """Host-side: pad + layout-transform into per-core in_maps."""
    x = np.ascontiguousarray(x, dtype=np.float32)
    weight = np.asarray(weight, dtype=np.float32)
    if MM_MODE == "f32r":
        x = round_fp32r(x)
        weight = round_fp32r(weight)
    in_np = np.float32
    if MM_MODE == "bf16":
        import ml_dtypes

        in_np = ml_dtypes.bfloat16
        x = x.astype(in_np)
        weight = weight.astype(in_np)
    # [core, C, img, HP, WP] zero-padded
    xp = np.zeros((N_CORES, IN_C, IMGS, HP, WP), dtype=in_np)
    xt = x.reshape(N_CORES, IMGS, IN_C, H, W).transpose(0, 2, 1, 3, 4)
    xp[:, :, :, PAD : PAD + H, PAD : PAD + W] = xt
    # weight (OUT_C, IN_C, K, K) -> [IN_C, HALVES, K*K, 128]
    wt = np.ascontiguousarray(
        weight.transpose(1, 2, 3, 0)           # [IN_C, K, K, OUT_C]
        .reshape(IN_C, K * K, HALVES, 128)
        .transpose(0, 2, 1, 3)                 # [IN_C, HALVES, K*K, 128]
    )
    # bias (256,) -> [128, 2] with bs[p, half] = bias[half*128 + p]
    bs = np.ascontiguousarray(
        np.asarray(bias, dtype=np.float32).reshape(HALVES, 128).T
    )
    return [
        {"xp": np.ascontiguousarray(xp[c]), "wt": wt, "bs": bs}
        for c in range(N_CORES)
    ]


def unshard_output(results):
    """[core][out: (2,128,4,3136)] -> (32,256,56,56)."""
    o = np.stack([r["out"] for r in results])  # [8, 2, 128, 4, 3136]
    return np.ascontiguousarray(
        o.transpose(0, 3, 1, 2, 4).reshape(N, OUT_C, H, W)
    )


def kernel(x: np.ndarray, weight: np.ndarray, bias: np.ndarray) -> np.ndarray:
    nc = build_nc()
    in_maps = shard_inputs(x, weight, bias)
    res = run_bass_kernel_spmd(nc, in_maps, core_ids=list(range(N_CORES)))
    return unshard_output(res.results)
